# revision 20
# baseline (speedup 1.0000x reference)
"""ConvNAT (conv QKV + 2D dilated neighborhood attention) on 8 trn2 cores.

Sharding: core = (batch b, H-slab of 24 rows).  Each core computes conv
q/k/v for the 36 kv rows its attention actually reads (image rows
h0-6 .. h0+29) and the attention for its 24 output rows.  H-border rows
(h<6, h>=90), whose NATTEN windows are clamped, are computed on the host
and overwrite the device output.

Attention on device (transposed-logits form):
  For each key row r (36 of them): L^T[kc, (j,q)] = K_ext_r^T Q_ext for
  all query rows j that attend to r (<=7, batched in one matmul via a
  strided moving AP).  Q/K_ext = [conv channels (64, bf16) | 7
  "h-distance" channels reproducing scale*pe_h.pe_h(h-h')], q side
  pre-scaled.  DVE adds WBT[kc, q] = scale*pe_w.pe_w + NEG mask
  (transposed W-bias table), ACT exps to bf16 P^T directly -- no PE
  transposes of P needed.
  AV per query row j: psum[96q, 129] = sum_i P_i^T^T vt_aug[r_i] where
  vt_aug rows carry [V^T | 1]; column 128 accumulates the softmax
  denominator.  DVE multiplies by 1/den during psum evacuation.
"""
import os
import re
import sys

sys.path.insert(0, '/opt/trn_rl_repo')

import numpy as np
import ml_dtypes

import concourse.bass as bass
from concourse import mybir
from concourse.tile import TileContext
from concourse.masks import make_identity
from bass_rust import ScopedClock, VectorClock

F32 = mybir.dt.float32
F32R = mybir.dt.float32r
BF16 = mybir.dt.bfloat16
NPBF16 = ml_dtypes.bfloat16

B, CIN, H, W = 2, 64, 96, 96
CI, CO = 64, 128
KS, DIL = 7, 2
SCALE = float(CI * 2) ** -0.5  # Cqk = 128 after pe concat
HS = 24          # rows per core
NH = 4           # h-slabs
NCORES = 8
NDIST = 7        # h-distance channels
CH = 64 + NDIST  # contraction channels
KV = 36          # kv rows per core (24 + 6 halo each side; row r = img h0-6+r)
XR = 38          # x slab rows (KV + conv halo; row t = img h0-7+t)
QOFF = 6         # q row j = kv row j+6
NEG = -30000.0
RING = 14        # pexp ring depth (AV(j) reads key rows j..j+12)
AVW = 130        # AV moving width: 128 v + ones col + pad (even for bf16)
VTW = 136        # vt row stride (128 v + 1 ones + pad)

# ---------------------------------------------------------------- compat ---
MAX_WAITS = 1


def _patched_drain(self, tick_clock, wait_clock):
    nc = self.nc
    ticks = [int(v) for v in re.findall(r'\d+', repr(tick_clock.global_clock))]
    for i in range(0, len(ticks), MAX_WAITS):
        chunk = [0] * len(ticks)
        chunk[i:i + MAX_WAITS] = ticks[i:i + MAX_WAITS]
        if any(chunk):
            probe = nc.sync.nop()
            wait_clock.add_sem_waits(probe.ins, ScopedClock({None: VectorClock(chunk)}))
    nc.sync.drain()
    nc.all_engine_barrier()
    popped = nc._tile_sem_poison_stack.pop()
    assert popped is self._sem_poison
    nc.clear_and_free_semaphores(list(self.sems.allocated().values()))
    nc.all_engine_barrier()


TileContext._drain_and_barrier = _patched_drain


def _split_excess_waits(nc, max_waits=MAX_WAITS):
    n_split = 0
    for fn in nc.m.functions:
        for bb in fn.blocks:
            out = []
            changed = False
            for inst in bb.instructions:
                si = inst.sync_info
                waits = list(si.on_wait) if si and si.on_wait else []
                if len(waits) > max_waits:
                    extra = waits[:-max_waits]
                    for j in range(0, len(extra), max_waits):
                        nop = mybir.InstNoOp(name=f"{inst.name}-ws{j}", ins=[], outs=[])
                        nop.engine = inst.engine
                        nop.sync_info = mybir.SyncInfo(
                            on_wait=extra[j:j + max_waits], on_update=[])
                        out.append(nop)
                    si.on_wait = waits[-max_waits:]
                    changed = True
                    n_split += 1
                out.append(inst)
            if changed:
                bb.instructions = out
    return n_split


# ------------------------------------------------------------- host math ---
def _sincos(length, dim):
    half = dim // 2
    inv_freq = 1.0 / (10000.0 ** (np.arange(half, dtype=np.float64) * 2.0 / dim))
    ang = np.arange(length, dtype=np.float64)[:, None] * inv_freq[None, :]
    return np.concatenate([np.sin(ang), np.cos(ang)], axis=-1)  # (L, dim)


def _na_indices(L, K, D):
    i = np.arange(L)
    g = i % D
    r = i // D
    Lg = (L - g + D - 1) // D
    start = np.clip(r - K // 2, 0, Lg - K)
    return g[:, None] + (start[:, None] + np.arange(K)[None, :]) * D  # (L, K)


def _hdist_channels():
    """QD,KD (NDIST, 96): sum_m QD[m,h]*KD[m,h'] == SCALE*pe_h[h].pe_h[h']
    exactly for even |h-h'| <= 6."""
    pe = _sincos(H, 32)           # (96, 32)
    inv_freq = 1.0 / (10000.0 ** (np.arange(16, dtype=np.float64) * 2.0 / 32))
    dv = np.array([0., 2., 4., 6.])
    g = SCALE * np.cos(dv[:, None] * inv_freq[None, :]).sum(1)  # exact pe.pe(d)
    th = np.arange(4, dtype=np.float64) * (np.pi / 6.0)
    M = np.cos(dv[:, None] * th[None, :])                        # (4, 4)
    b = np.linalg.solve(M, g)
    hh = np.arange(H, dtype=np.float64)
    QD = np.zeros((NDIST, H))
    KD = np.zeros((NDIST, H))
    QD[0] = b[0]
    KD[0] = 1.0
    for m in range(1, 4):
        QD[2 * m - 1] = b[m] * np.cos(th[m] * hh)
        QD[2 * m] = b[m] * np.sin(th[m] * hh)
        KD[2 * m - 1] = np.cos(th[m] * hh)
        KD[2 * m] = np.sin(th[m] * hh)
    # verify
    got = QD.T @ KD
    ref = SCALE * (pe @ pe.T)
    for dd in (-6, -4, -2, 0, 2, 4, 6):
        idx = np.arange(max(0, -dd), min(H, H - dd))
        err = np.abs(got[idx, idx + dd] - ref[idx, idx + dd]).max()
        assert err < 1e-6, (dd, err)
    return QD.astype(np.float32), KD.astype(np.float32)


def _wbias():
    pe = _sincos(W, 32)
    idx_w = _na_indices(W, KS, DIL)   # (96, 7)
    wb = np.full((W, W), NEG, dtype=np.float64)
    dot = SCALE * (pe @ pe.T)
    for w in range(W):
        wb[w, idx_w[w]] = dot[w, idx_w[w]]
    return wb.astype(np.float32)


def _conv_np(x, w, bias, rows):
    """NCHW 3x3 pad-1 conv evaluated at `rows` -> (B, len(rows), 96, Cout)."""
    Bn, Cin, Hn, Wn = x.shape
    xp = np.zeros((Bn, Cin, Hn + 2, Wn + 2), dtype=np.float64)
    xp[:, :, 1:-1, 1:-1] = x
    rows = np.asarray(rows)
    acc = np.zeros((Bn, len(rows), Wn, w.shape[0]), dtype=np.float64)
    for ky in range(3):
        for kx in range(3):
            xs = xp[:, :, rows + ky, :][:, :, :, kx:kx + Wn]  # (B,C,R,W)
            acc += np.einsum('bcrw,oc->brwo', xs, w[:, :, ky, kx].astype(np.float64))
    return acc + bias[None, None, None, :].astype(np.float64)


def _host_border(x, wq, bq, wk, bk, wv, bv):
    """Reference computation for the clamped border rows. -> dict h -> (B,96,128)."""
    border_h = list(range(0, 6)) + list(range(90, 96))
    kv_rows = sorted(set(np.concatenate([_na_indices(H, KS, DIL)[h] for h in border_h])))
    kv_rows = np.asarray(kv_rows)
    q_c = _conv_np(x, wq, bq, np.asarray(border_h))     # (B, 12, 96, 64)
    k_c = _conv_np(x, wk, bk, kv_rows)                  # (B, R, 96, 64)
    v_c = _conv_np(x, wv, bv, kv_rows)                  # (B, R, 96, 128)
    kv_pos = {r: i for i, r in enumerate(kv_rows)}
    pe_h = _sincos(H, 32)
    pe_w = _sincos(W, 32)
    idx_h = _na_indices(H, KS, DIL)
    idx_w = _na_indices(W, KS, DIL)
    out = {}
    for bi, h in enumerate(border_h):
        pe_q = np.concatenate([np.repeat(pe_h[h][None], W, 0), pe_w], axis=1)  # (96,64)
        q = np.concatenate([q_c[:, bi], np.repeat(pe_q[None], B, 0)], axis=2)  # (B,96,128)
        rows = [kv_pos[r] for r in idx_h[h]]
        kk = k_c[:, rows]                                   # (B,7,96,64)
        vv = v_c[:, rows]                                   # (B,7,96,128)
        pe_k = np.concatenate(
            [np.repeat(pe_h[idx_h[h]][:, None, :], W, 1),
             np.repeat(pe_w[None], KS, 0)], axis=2)         # (7,96,64)
        kk = np.concatenate([kk, np.repeat(pe_k[None], B, 0)], axis=3)  # (B,7,96,128)
        kn = kk[:, :, idx_w]                                # (B,7,96,7,128)
        vn = vv[:, :, idx_w]
        logits = SCALE * np.einsum('bwc,biwjc->bwij', q, kn)   # (B,96,7,7)
        m = logits.reshape(B, W, -1).max(-1)
        p = np.exp(logits - m[:, :, None, None])
        p /= p.reshape(B, W, -1).sum(-1)[:, :, None, None]
        out[h] = np.einsum('bwij,biwjc->bwc', p, vn)        # (B,96,128)
    return out


def _users(r):
    """Query rows j (0..23) attending key row r: j = r-2i, i=0..6."""
    j0 = r - 12 if r >= 12 else r % 2
    j1 = min(r, 23)
    if j0 > j1:
        return j0, 0
    return j0, (j1 - j0) // 2 + 1


# ------------------------------------------------------------ bass build ---
_CACHE = {}


def _build_program():
    if 'nc' in _CACHE:
        return _CACHE['nc']
    nc = bass.Bass('TRN2')
    xs = nc.dram_tensor('xs', (64, XR, 98), BF16, kind='ExternalInput')
    wqk2 = nc.dram_tensor('wqk2', (128, 3, 128), BF16, kind='ExternalInput')
    wqk1 = nc.dram_tensor('wqk1', (64, 3, 128), BF16, kind='ExternalInput')
    wv2 = nc.dram_tensor('wv2', (128, 3, 128), BF16, kind='ExternalInput')
    wv1 = nc.dram_tensor('wv1', (64, 3, 128), BF16, kind='ExternalInput')
    qkb = nc.dram_tensor('qkb', (128, 1), F32, kind='ExternalInput')
    vb = nc.dram_tensor('vb', (128, 1), F32, kind='ExternalInput')
    qd = nc.dram_tensor('qd', (NDIST, HS * 96), BF16, kind='ExternalInput')
    kd = nc.dram_tensor('kd', (NDIST, KV * 96), BF16, kind='ExternalInput')
    wbt = nc.dram_tensor('wbt', (96, 96), F32, kind='ExternalInput')
    o = nc.dram_tensor('o', (HS, 96, 128), F32, kind='ExternalOutput')

    with TileContext(nc) as tc:
        with tc.tile_pool(name='persist', bufs=1) as pp:
            # ---- input DMAs spread over queues; x in 3 row-chunks so conv
            # can start on chunk 0 while later chunks stream in.
            w_qk2 = pp.tile([128, 3, 128], BF16)
            nc.scalar.dma_start(out=w_qk2, in_=wqk2[:])
            # ky=2 weights: kx0/kx1 stacked as one 128-contract pair + kx2
            w_qk1p = pp.tile([128, 128], BF16)
            nc.scalar.dma_start(out=w_qk1p[0:64, :], in_=wqk1[:, 0, :])
            nc.scalar.dma_start(out=w_qk1p[64:128, :], in_=wqk1[:, 1, :])
            w_qk1s = pp.tile([64, 128], BF16)
            nc.scalar.dma_start(out=w_qk1s, in_=wqk1[:, 2, :])

            # x in bf16, three copies so every moving AP has an even element
            # offset and the ky=2 taps pair up to a full 128 contraction:
            #   x2  [row t | row t+1] cols 0..97   (ky01 kx=0 @0, kx=2 @2,
            #                                       ky2 kx=2 on low half @2)
            #   x2b [row t | row t+1] cols 1..96   (ky01 kx=1 @0)
            #   xw  [cols c | cols c+1] same row   (ky2 kx=0+kx=1 pair @0)
            x2 = pp.tile([128, XR, 98], BF16)
            x2b = pp.tile([128, XR, 96], BF16)
            xw = pp.tile([128, XR, 96], BF16)
            for (a, bnd) in ((0, 9), (9, 19), (19, 28), (28, XR)):
                nc.sync.dma_start(out=x2[0:64, a:bnd, :], in_=xs[:, a:bnd, :])
                nc.scalar.dma_start(out=x2b[0:64, a:bnd, :],
                                    in_=xs[:, a:bnd, 1:97])
                b2 = min(bnd, XR - 1)
                nc.gpsimd.dma_start(out=x2[64:128, a:b2, :],
                                    in_=xs[:, a + 1:b2 + 1, :])
                nc.sync.dma_start(out=xw[0:64, a:bnd, :],
                                  in_=xs[:, a:bnd, 0:96])
                nc.scalar.dma_start(out=xw[64:128, a:bnd, :],
                                    in_=xs[:, a:bnd, 1:97])
                nc.gpsimd.dma_start(out=x2b[64:128, a:b2, :],
                                    in_=xs[:, a + 1:b2 + 1, 1:97])

            w_v2 = pp.tile([128, 3, 128], BF16)
            nc.scalar.dma_start(out=w_v2, in_=wv2[:])
            w_v1p = pp.tile([128, 128], BF16)
            nc.scalar.dma_start(out=w_v1p[0:64, :], in_=wv1[:, 0, :])
            nc.scalar.dma_start(out=w_v1p[64:128, :], in_=wv1[:, 1, :])
            w_v1s = pp.tile([64, 128], BF16)
            nc.scalar.dma_start(out=w_v1s, in_=wv1[:, 2, :])

            qkbias = pp.tile([128, 1], F32)
            nc.gpsimd.dma_start(out=qkbias, in_=qkb[:])
            vbias = pp.tile([128, 1], F32)
            nc.gpsimd.dma_start(out=vbias, in_=vb[:])
            wbT = pp.tile([96, 96], F32)
            nc.gpsimd.dma_start(out=wbT, in_=wbt[:])
            ident = pp.tile([128, 128], BF16)
            make_identity(nc, ident)

            q_ext = pp.tile([CH, HS * 96], BF16)
            k_ext = pp.tile([CH, KV * 96], BF16)
            vsb = pp.tile([128, KV * 96], BF16)
            vt = pp.tile([96, KV, VTW], BF16)
            nc.sync.dma_start(out=q_ext[64:CH, :], in_=qd[:])
            nc.gpsimd.dma_start(out=k_ext[64:CH, :], in_=kd[:])
            nc.gpsimd.memset(vt[:, :, 128:130], 1.0)

            # ------------------------------------------------ convolution --
            # kv row r (img h0-6+r) from x slab rows r,r+1,r+2 (ky=0,1,2);
            # ky=0,1 fused via the doubled-x tile, ky=2 as K=64 matmuls.
            # 9 groups of 4 rows per layer; one psum pool (bufs=6) pipelines
            # groups across waves without pool-transition barriers.
            GRP = ((0, 5), (5, 10), (10, 15), (15, 20),
                   (20, 24), (24, 28), (28, 32), (32, 36))
            with tc.tile_pool(name='cps', bufs=6, space='PSUM') as cps:
                for which in range(2):  # 0 = qk, 1 = v
                    w2 = w_qk2 if which == 0 else w_v2
                    w1p = w_qk1p if which == 0 else w_v1p
                    w1s = w_qk1s if which == 0 else w_v1s
                    for (ga, gb) in ((0, 4), (4, 8)):
                        psl = {g: cps.tile([128, 5, 96], F32, tag='c',
                                           name=f'c{which}_{g}')
                               for g in range(ga, gb)}
                        for tap in range(5):
                            for g in range(ga, gb):
                                r0, r1 = GRP[g]
                                sz = r1 - r0
                                if tap == 0:
                                    nc.tensor.matmul(
                                        psl[g][:, 0:sz, :], w2[:, 0, :],
                                        x2[:, r0:r1, 0:96],
                                        start=True, stop=False)
                                elif tap == 1:
                                    nc.tensor.matmul(
                                        psl[g][:, 0:sz, :], w2[:, 1, :],
                                        x2b[:, r0:r1, 0:96],
                                        start=False, stop=False)
                                elif tap == 2:
                                    nc.tensor.matmul(
                                        psl[g][:, 0:sz, :], w2[:, 2, :],
                                        x2[:, r0:r1, 2:98],
                                        start=False, stop=False)
                                elif tap == 3:
                                    nc.tensor.matmul(
                                        psl[g][:, 0:sz, :], w1p,
                                        xw[:, r0 + 2:r1 + 2, 0:96],
                                        start=False, stop=False)
                                else:
                                    nc.tensor.matmul(
                                        psl[g][:, 0:sz, :], w1s,
                                        x2[0:64, r0 + 2:r1 + 2, 2:98],
                                        start=False, stop=True)
                        for g in range(ga, gb):
                            r0, r1 = GRP[g]
                            sz = r1 - r0
                            if which == 0:
                                # k rows: all; q rows: kv rows 6..29 only
                                nc.vector.tensor_scalar(
                                    out=k_ext[0:64, r0 * 96:r1 * 96],
                                    in0=psl[g][64:128, 0:sz, :],
                                    scalar1=qkbias[64:128],
                                    scalar2=None, op0=mybir.AluOpType.add)
                                qa = max(r0, QOFF) - r0
                                qb = min(r1, QOFF + HS) - r0
                                if qa < qb:
                                    nc.scalar.activation(
                                        out=q_ext[0:64, (r0 + qa - QOFF) * 96:
                                                  (r0 + qb - QOFF) * 96],
                                        in_=psl[g][0:64, qa:qb, :],
                                        func=mybir.ActivationFunctionType.Identity,
                                        bias=qkbias[0:64])
                            else:
                                if g % 2 == 0:
                                    nc.scalar.activation(
                                        out=vsb[:, r0 * 96:r1 * 96],
                                        in_=psl[g][:, 0:sz, :],
                                        func=mybir.ActivationFunctionType.Identity,
                                        bias=vbias[:])
                                else:
                                    nc.vector.tensor_scalar(
                                        out=vsb[:, r0 * 96:r1 * 96],
                                        in0=psl[g][:, 0:sz, :],
                                        scalar1=vbias[:],
                                        scalar2=None, op0=mybir.AluOpType.add)

            # ----------------------------------- V^T + attention ----------
            wb_ap = wbT[:, :]

            with tc.tile_pool(name='tps', bufs=2, space='PSUM') as tps, \
                 tc.tile_pool(name='lps', bufs=2, space='PSUM') as lps, \
                 tc.tile_pool(name='ops', bufs=2, space='PSUM') as ops, \
                 tc.tile_pool(name='pxp', bufs=RING) as pxp, \
                 tc.tile_pool(name='att', bufs=3) as att:
                pexp = {}

                def vtrans(grp):
                    pst = tps.tile([96, 4, 128], BF16, tag='t')
                    for rr in range(4):
                        r = grp * 4 + rr
                        nc.tensor.transpose(
                            pst[:, rr, :], vsb[:, r * 96:(r + 1) * 96], ident)
                    nc.vector.tensor_copy(
                        out=vt[:, grp * 4:grp * 4 + 4, 0:128], in_=pst[:])

                def logits(r):
                    j0, n = _users(r)
                    # rows padded to 128 f32 so 4 rows = one PSUM bank
                    # (matmul output cannot cross a bank boundary)
                    psL = lps.tile([96, 7, 128], F32, tag='L')
                    ke = k_ext[:, r * 96:(r + 1) * 96]
                    for (u0, u1) in ((0, min(n, 4)), (4, n)):
                        if u0 >= u1:
                            continue
                        qa = q_ext[:, (j0 + 2 * u0) * 96:(j0 + 2 * u0 + 1) * 96]
                        rhs = bass.AP(tensor=qa.tensor, offset=qa.offset,
                                      ap=[qa.ap[0], [2 * 96, u1 - u0], qa.ap[1]])
                        nc.tensor.matmul(psL[:, u0:u1, 0:96], ke, rhs,
                                         start=True, stop=True)
                    lm = att.tile([96, 7, 96], F32, tag='lm')
                    wb_b = bass.AP(tensor=wb_ap.tensor, offset=wb_ap.offset,
                                   ap=[wb_ap.ap[0], [0, n], wb_ap.ap[1]])
                    nc.vector.tensor_tensor(out=lm[:, 0:n, :],
                                            in0=psL[:, 0:n, 0:96],
                                            in1=wb_b, op=mybir.AluOpType.add)
                    px = pxp.tile([96, 7, 96], BF16, tag='px')
                    nc.scalar.activation(out=px[:, 0:n, :], in_=lm[:, 0:n, :],
                                         func=mybir.ActivationFunctionType.Exp)
                    pexp[r] = px

                def av(j):
                    psO = ops.tile([96, AVW], F32, tag='O')
                    for i in range(KS):
                        r = j + 2 * i
                        j0, _ = _users(r)
                        slot = (j - j0) // 2
                        nc.tensor.matmul(psO[:],
                                         pexp[r][:, slot, :],
                                         vt[:, r, 0:AVW],
                                         start=(i == 0), stop=(i == KS - 1))
                    rden = att.tile([96, 1], F32, tag='rden')
                    nc.vector.reciprocal(out=rden[:], in_=psO[:, 128:129])
                    oh = att.tile([96, 128], F32, tag='oh')
                    nc.vector.tensor_scalar_mul(oh[:], psO[:, 0:128], rden[:])
                    (nc.sync if j % 2 == 0 else nc.gpsimd).dma_start(
                        out=o[j], in_=oh[:])

                # interleave V^T transposes with the logits prologue so the
                # bias/exp pipeline is primed when the AV loop starts
                for grp in range(9):
                    vtrans(grp)
                    for r in ((2 * grp, 2 * grp + 1) if grp < 4 else
                              (grp + 4,)):
                        logits(r)
                for j in range(HS):
                    av(j)
                    if j + 13 < KV:
                        logits(j + 13)

    _split_excess_waits(nc)
    _CACHE['nc'] = nc
    return nc


# ---------------------------------------------------------------- kernel ---
def _make_in_maps(x, wq, bq, wk, bk, wv, bv):
    QD, KD = _hdist_channels()
    wbias = _wbias()
    wq_s = wq * SCALE
    w2 = np.zeros((3, 128, 128), dtype=np.float32)
    w1 = np.zeros((3, 64, 128), dtype=np.float32)
    v2 = np.zeros((3, 128, 128), dtype=np.float32)
    v1 = np.zeros((3, 64, 128), dtype=np.float32)
    for kx in range(3):
        w2[kx, 0:64, 0:64] = wq_s[:, :, 0, kx].T
        w2[kx, 0:64, 64:128] = wk[:, :, 0, kx].T
        w2[kx, 64:128, 0:64] = wq_s[:, :, 1, kx].T
        w2[kx, 64:128, 64:128] = wk[:, :, 1, kx].T
        w1[kx, :, 0:64] = wq_s[:, :, 2, kx].T
        w1[kx, :, 64:128] = wk[:, :, 2, kx].T
        v2[kx, 0:64, :] = wv[:, :, 0, kx].T
        v2[kx, 64:128, :] = wv[:, :, 1, kx].T
        v1[kx, :, :] = wv[:, :, 2, kx].T
    qkbias = np.concatenate([bq * SCALE, bk]).reshape(128, 1).astype(np.float32)
    vbias = bv.reshape(128, 1).astype(np.float32)

    in_maps = []
    for core in range(NCORES):
        b, slab = core // NH, core % NH
        h0 = slab * HS
        xsl = np.zeros((64, XR, 98), dtype=np.float32)
        r_lo, r_hi = h0 - 7, h0 - 7 + XR  # image rows of x slab
        src_lo, src_hi = max(0, r_lo), min(H, r_hi)
        xsl[:, src_lo - r_lo: src_hi - r_lo, 1:97] = x[b, :, src_lo:src_hi, :]
        qdf = np.repeat(QD[:, h0:h0 + HS, None], 96, axis=2).reshape(NDIST, -1)
        kdf = np.zeros((NDIST, KV, 96), dtype=np.float32)
        for r in range(KV):
            img = h0 - QOFF + r
            if 0 <= img < H:
                kdf[:, r, :] = KD[:, img, None]
        in_maps.append({
            'xs': xsl.astype(NPBF16),
            'wqk2': np.ascontiguousarray(w2.transpose(1, 0, 2)).astype(NPBF16),
            'wqk1': np.ascontiguousarray(w1.transpose(1, 0, 2)).astype(NPBF16),
            'wv2': np.ascontiguousarray(v2.transpose(1, 0, 2)).astype(NPBF16),
            'wv1': np.ascontiguousarray(v1.transpose(1, 0, 2)).astype(NPBF16),
            'qkb': qkbias, 'vb': vbias,
            'qd': np.ascontiguousarray(qdf).astype(NPBF16),
            'kd': np.ascontiguousarray(kdf.reshape(NDIST, -1)).astype(NPBF16),
            'wbt': np.ascontiguousarray(wbias.T),
        })
    return in_maps


def kernel(x, wq, bq, wk, bk, wv, bv):
    x = np.asarray(x, dtype=np.float32)
    wq = np.asarray(wq, dtype=np.float32)
    wk = np.asarray(wk, dtype=np.float32)
    wv = np.asarray(wv, dtype=np.float32)
    bq = np.asarray(bq, dtype=np.float32)
    bk = np.asarray(bk, dtype=np.float32)
    bv = np.asarray(bv, dtype=np.float32)

    nc = _build_program()
    in_maps = _make_in_maps(x=x, wq=wq, bq=bq, wk=wk, bk=bk, wv=wv, bv=bv)

    from concourse.bass_utils import run_bass_kernel_spmd
    res = run_bass_kernel_spmd(nc, in_maps, core_ids=list(range(NCORES)))
    global LAST_RESULT
    LAST_RESULT = res

    out = np.zeros((B, H, W, CO), dtype=np.float32)
    for core in range(NCORES):
        b, slab = core // NH, core % NH
        out[b, slab * HS:(slab + 1) * HS] = res.results[core]['o']

    border = _host_border(x, wq, bq, wk, bk, wv, bv)
    for h, val in border.items():
        out[:, h] = val.astype(np.float32)
    return out


# revision 21
# speedup vs baseline: 1.1865x; 1.1865x over previous
"""ConvNAT (conv QKV + 2D dilated neighborhood attention) on 8 trn2 cores.

Sharding: core = (batch b, H-slab of 24 rows).  Each core computes conv
q/k/v for the 36 kv rows its attention actually reads (image rows
h0-6 .. h0+29) and the attention for its 24 output rows.  H-border rows
(h<6, h>=90), whose NATTEN windows are clamped, are computed on the host
and overwrite the device output.

Attention on device (transposed-logits form):
  For each key row r (36 of them): L^T[kc, (j,q)] = K_ext_r^T Q_ext for
  all query rows j that attend to r (<=7, batched in one matmul via a
  strided moving AP).  Q/K_ext = [conv channels (64, bf16) | 7
  "h-distance" channels reproducing scale*pe_h.pe_h(h-h')], q side
  pre-scaled.  DVE adds WBT[kc, q] = scale*pe_w.pe_w + NEG mask
  (transposed W-bias table), ACT exps to bf16 P^T directly -- no PE
  transposes of P needed.
  AV per query row j: psum[96q, 129] = sum_i P_i^T^T vt_aug[r_i] where
  vt_aug rows carry [V^T | 1]; column 128 accumulates the softmax
  denominator.  DVE multiplies by 1/den during psum evacuation.
"""
import os
import re
import sys

sys.path.insert(0, '/opt/trn_rl_repo')

import numpy as np
import ml_dtypes

import concourse.bass as bass
from concourse import mybir
from concourse.tile import TileContext
from concourse.masks import make_identity
from bass_rust import ScopedClock, VectorClock

F32 = mybir.dt.float32
F32R = mybir.dt.float32r
BF16 = mybir.dt.bfloat16
NPBF16 = ml_dtypes.bfloat16

B, CIN, H, W = 2, 64, 96, 96
CI, CO = 64, 128
KS, DIL = 7, 2
SCALE = float(CI * 2) ** -0.5  # Cqk = 128 after pe concat
HS = 24          # rows per core
NH = 4           # h-slabs
NCORES = 8
NDIST = 7        # h-distance channels
CH = 64 + NDIST  # contraction channels
KV = 36          # kv rows per core (24 + 6 halo each side; row r = img h0-6+r)
XR = 38          # x slab rows (KV + conv halo; row t = img h0-7+t)
QOFF = 6         # q row j = kv row j+6
NEG = -30000.0
RING = 14        # pexp ring depth (AV(j) reads key rows j..j+12)
AVW = 130        # AV moving width: 128 v + ones col + pad (even for bf16)
VTW = 136        # vt row stride (128 v + 1 ones + pad)

# ---------------------------------------------------------------- compat ---
MAX_WAITS = 1


def _patched_drain(self, tick_clock, wait_clock):
    nc = self.nc
    ticks = [int(v) for v in re.findall(r'\d+', repr(tick_clock.global_clock))]
    for i in range(0, len(ticks), MAX_WAITS):
        chunk = [0] * len(ticks)
        chunk[i:i + MAX_WAITS] = ticks[i:i + MAX_WAITS]
        if any(chunk):
            probe = nc.sync.nop()
            wait_clock.add_sem_waits(probe.ins, ScopedClock({None: VectorClock(chunk)}))
    nc.sync.drain()
    nc.all_engine_barrier()
    popped = nc._tile_sem_poison_stack.pop()
    assert popped is self._sem_poison
    nc.clear_and_free_semaphores(list(self.sems.allocated().values()))
    nc.all_engine_barrier()


TileContext._drain_and_barrier = _patched_drain


def _split_excess_waits(nc, max_waits=MAX_WAITS):
    n_split = 0
    for fn in nc.m.functions:
        for bb in fn.blocks:
            out = []
            changed = False
            for inst in bb.instructions:
                si = inst.sync_info
                waits = list(si.on_wait) if si and si.on_wait else []
                if len(waits) > max_waits:
                    extra = waits[:-max_waits]
                    for j in range(0, len(extra), max_waits):
                        nop = mybir.InstNoOp(name=f"{inst.name}-ws{j}", ins=[], outs=[])
                        nop.engine = inst.engine
                        nop.sync_info = mybir.SyncInfo(
                            on_wait=extra[j:j + max_waits], on_update=[])
                        out.append(nop)
                    si.on_wait = waits[-max_waits:]
                    changed = True
                    n_split += 1
                out.append(inst)
            if changed:
                bb.instructions = out
    return n_split


# ------------------------------------------------------------- host math ---
def _sincos(length, dim):
    half = dim // 2
    inv_freq = 1.0 / (10000.0 ** (np.arange(half, dtype=np.float64) * 2.0 / dim))
    ang = np.arange(length, dtype=np.float64)[:, None] * inv_freq[None, :]
    return np.concatenate([np.sin(ang), np.cos(ang)], axis=-1)  # (L, dim)


def _na_indices(L, K, D):
    i = np.arange(L)
    g = i % D
    r = i // D
    Lg = (L - g + D - 1) // D
    start = np.clip(r - K // 2, 0, Lg - K)
    return g[:, None] + (start[:, None] + np.arange(K)[None, :]) * D  # (L, K)


def _hdist_channels():
    """QD,KD (NDIST, 96): sum_m QD[m,h]*KD[m,h'] == SCALE*pe_h[h].pe_h[h']
    exactly for even |h-h'| <= 6."""
    pe = _sincos(H, 32)           # (96, 32)
    inv_freq = 1.0 / (10000.0 ** (np.arange(16, dtype=np.float64) * 2.0 / 32))
    dv = np.array([0., 2., 4., 6.])
    g = SCALE * np.cos(dv[:, None] * inv_freq[None, :]).sum(1)  # exact pe.pe(d)
    th = np.arange(4, dtype=np.float64) * (np.pi / 6.0)
    M = np.cos(dv[:, None] * th[None, :])                        # (4, 4)
    b = np.linalg.solve(M, g)
    hh = np.arange(H, dtype=np.float64)
    QD = np.zeros((NDIST, H))
    KD = np.zeros((NDIST, H))
    QD[0] = b[0]
    KD[0] = 1.0
    for m in range(1, 4):
        QD[2 * m - 1] = b[m] * np.cos(th[m] * hh)
        QD[2 * m] = b[m] * np.sin(th[m] * hh)
        KD[2 * m - 1] = np.cos(th[m] * hh)
        KD[2 * m] = np.sin(th[m] * hh)
    # verify
    got = QD.T @ KD
    ref = SCALE * (pe @ pe.T)
    for dd in (-6, -4, -2, 0, 2, 4, 6):
        idx = np.arange(max(0, -dd), min(H, H - dd))
        err = np.abs(got[idx, idx + dd] - ref[idx, idx + dd]).max()
        assert err < 1e-6, (dd, err)
    return QD.astype(np.float32), KD.astype(np.float32)


def _wbias():
    pe = _sincos(W, 32)
    idx_w = _na_indices(W, KS, DIL)   # (96, 7)
    wb = np.full((W, W), NEG, dtype=np.float64)
    dot = SCALE * (pe @ pe.T)
    for w in range(W):
        wb[w, idx_w[w]] = dot[w, idx_w[w]]
    return wb.astype(np.float32)


def _conv_np(x, w, bias, rows):
    """NCHW 3x3 pad-1 conv evaluated at `rows` -> (B, len(rows), 96, Cout)."""
    Bn, Cin, Hn, Wn = x.shape
    xp = np.zeros((Bn, Cin, Hn + 2, Wn + 2), dtype=np.float64)
    xp[:, :, 1:-1, 1:-1] = x
    rows = np.asarray(rows)
    acc = np.zeros((Bn, len(rows), Wn, w.shape[0]), dtype=np.float64)
    for ky in range(3):
        for kx in range(3):
            xs = xp[:, :, rows + ky, :][:, :, :, kx:kx + Wn]  # (B,C,R,W)
            acc += np.einsum('bcrw,oc->brwo', xs, w[:, :, ky, kx].astype(np.float64))
    return acc + bias[None, None, None, :].astype(np.float64)


def _host_border(x, wq, bq, wk, bk, wv, bv):
    """Reference computation for the clamped border rows. -> dict h -> (B,96,128)."""
    border_h = list(range(0, 6)) + list(range(90, 96))
    kv_rows = sorted(set(np.concatenate([_na_indices(H, KS, DIL)[h] for h in border_h])))
    kv_rows = np.asarray(kv_rows)
    q_c = _conv_np(x, wq, bq, np.asarray(border_h))     # (B, 12, 96, 64)
    k_c = _conv_np(x, wk, bk, kv_rows)                  # (B, R, 96, 64)
    v_c = _conv_np(x, wv, bv, kv_rows)                  # (B, R, 96, 128)
    kv_pos = {r: i for i, r in enumerate(kv_rows)}
    pe_h = _sincos(H, 32)
    pe_w = _sincos(W, 32)
    idx_h = _na_indices(H, KS, DIL)
    idx_w = _na_indices(W, KS, DIL)
    out = {}
    for bi, h in enumerate(border_h):
        pe_q = np.concatenate([np.repeat(pe_h[h][None], W, 0), pe_w], axis=1)  # (96,64)
        q = np.concatenate([q_c[:, bi], np.repeat(pe_q[None], B, 0)], axis=2)  # (B,96,128)
        rows = [kv_pos[r] for r in idx_h[h]]
        kk = k_c[:, rows]                                   # (B,7,96,64)
        vv = v_c[:, rows]                                   # (B,7,96,128)
        pe_k = np.concatenate(
            [np.repeat(pe_h[idx_h[h]][:, None, :], W, 1),
             np.repeat(pe_w[None], KS, 0)], axis=2)         # (7,96,64)
        kk = np.concatenate([kk, np.repeat(pe_k[None], B, 0)], axis=3)  # (B,7,96,128)
        kn = kk[:, :, idx_w]                                # (B,7,96,7,128)
        vn = vv[:, :, idx_w]
        logits = SCALE * np.einsum('bwc,biwjc->bwij', q, kn)   # (B,96,7,7)
        m = logits.reshape(B, W, -1).max(-1)
        p = np.exp(logits - m[:, :, None, None])
        p /= p.reshape(B, W, -1).sum(-1)[:, :, None, None]
        out[h] = np.einsum('bwij,biwjc->bwc', p, vn)        # (B,96,128)
    return out


def _users(r):
    """Query rows j (0..23) attending key row r: j = r-2i, i=0..6."""
    j0 = r - 12 if r >= 12 else r % 2
    j1 = min(r, 23)
    if j0 > j1:
        return j0, 0
    return j0, (j1 - j0) // 2 + 1


# ------------------------------------------------------------ bass build ---
_CACHE = {}


def _build_program():
    if 'nc' in _CACHE:
        return _CACHE['nc']
    nc = bass.Bass('TRN2')
    xs = nc.dram_tensor('xs', (64, XR, 98), BF16, kind='ExternalInput')
    wqk2 = nc.dram_tensor('wqk2', (128, 3, 128), BF16, kind='ExternalInput')
    wqk1 = nc.dram_tensor('wqk1', (64, 3, 128), BF16, kind='ExternalInput')
    wv2 = nc.dram_tensor('wv2', (128, 3, 128), BF16, kind='ExternalInput')
    wv1 = nc.dram_tensor('wv1', (64, 3, 128), BF16, kind='ExternalInput')
    qkb = nc.dram_tensor('qkb', (128, 1), F32, kind='ExternalInput')
    vb = nc.dram_tensor('vb', (128, 1), F32, kind='ExternalInput')
    qd = nc.dram_tensor('qd', (NDIST, HS * 96), BF16, kind='ExternalInput')
    kd = nc.dram_tensor('kd', (NDIST, KV * 96), BF16, kind='ExternalInput')
    wbt = nc.dram_tensor('wbt', (96, 96), F32, kind='ExternalInput')
    o = nc.dram_tensor('o', (HS, 96, 128), F32, kind='ExternalOutput')

    with TileContext(nc) as tc:
        with tc.tile_pool(name='persist', bufs=1) as pp:
            # ---- input DMAs spread over queues; x in 3 row-chunks so conv
            # can start on chunk 0 while later chunks stream in.
            w_qk2 = pp.tile([128, 3, 128], BF16)
            nc.scalar.dma_start(out=w_qk2, in_=wqk2[:])
            # ky=2 weights: kx0/kx1 stacked as one 128-contract pair + kx2
            w_qk1p = pp.tile([128, 128], BF16)
            nc.scalar.dma_start(out=w_qk1p[0:64, :], in_=wqk1[:, 0, :])
            nc.scalar.dma_start(out=w_qk1p[64:128, :], in_=wqk1[:, 1, :])
            w_qk1s = pp.tile([64, 128], BF16)
            nc.scalar.dma_start(out=w_qk1s, in_=wqk1[:, 2, :])

            # x in bf16, three copies so every moving AP has an even element
            # offset and the ky=2 taps pair up to a full 128 contraction:
            #   x2  [row t | row t+1] cols 0..97   (ky01 kx=0 @0, kx=2 @2,
            #                                       ky2 kx=2 on low half @2)
            #   x2b [row t | row t+1] cols 1..96   (ky01 kx=1 @0)
            #   xw  [cols c | cols c+1] same row   (ky2 kx=0+kx=1 pair @0)
            x2 = pp.tile([128, XR, 98], BF16)
            x2b = pp.tile([128, XR, 96], BF16)
            xw = pp.tile([128, XR, 96], BF16)
            for (a, bnd) in ((0, 9), (9, 19), (19, 28), (28, XR)):
                nc.sync.dma_start(out=x2[0:64, a:bnd, :], in_=xs[:, a:bnd, :])
                nc.scalar.dma_start(out=x2b[0:64, a:bnd, :],
                                    in_=xs[:, a:bnd, 1:97])
                b2 = min(bnd, XR - 1)
                nc.gpsimd.dma_start(out=x2[64:128, a:b2, :],
                                    in_=xs[:, a + 1:b2 + 1, :])
                nc.sync.dma_start(out=xw[0:64, a:bnd, :],
                                  in_=xs[:, a:bnd, 0:96])
                nc.scalar.dma_start(out=xw[64:128, a:bnd, :],
                                    in_=xs[:, a:bnd, 1:97])
                nc.gpsimd.dma_start(out=x2b[64:128, a:b2, :],
                                    in_=xs[:, a + 1:b2 + 1, 1:97])

            w_v2 = pp.tile([128, 3, 128], BF16)
            nc.scalar.dma_start(out=w_v2, in_=wv2[:])
            w_v1p = pp.tile([128, 128], BF16)
            nc.scalar.dma_start(out=w_v1p[0:64, :], in_=wv1[:, 0, :])
            nc.scalar.dma_start(out=w_v1p[64:128, :], in_=wv1[:, 1, :])
            w_v1s = pp.tile([64, 128], BF16)
            nc.scalar.dma_start(out=w_v1s, in_=wv1[:, 2, :])

            qkbias = pp.tile([128, 1], F32)
            nc.gpsimd.dma_start(out=qkbias, in_=qkb[:])
            vbias = pp.tile([128, 1], F32)
            nc.gpsimd.dma_start(out=vbias, in_=vb[:])
            wbT = pp.tile([96, 96], F32)
            nc.gpsimd.dma_start(out=wbT, in_=wbt[:])
            ident = pp.tile([128, 128], BF16)
            make_identity(nc, ident)

            q_ext = pp.tile([CH, HS * 96], BF16)
            k_ext = pp.tile([CH, KV * 96], BF16)
            vsb = pp.tile([128, KV * 96], BF16)
            vt = pp.tile([96, KV, VTW], BF16)
            nc.sync.dma_start(out=q_ext[64:CH, :], in_=qd[:])
            nc.gpsimd.dma_start(out=k_ext[64:CH, :], in_=kd[:])
            nc.gpsimd.memset(vt[:, :, 128:130], 1.0)

            # ------------------------------------------------ convolution --
            # kv row r (img h0-6+r) from x slab rows r,r+1,r+2 (ky=0,1,2);
            # ky=0,1 fused via the doubled-x tile, ky=2 as K=64 matmuls.
            # 9 groups of 4 rows per layer; one psum pool (bufs=6) pipelines
            # groups across waves without pool-transition barriers.
            GRP = ((0, 5), (5, 10), (10, 15), (15, 20),
                   (20, 24), (24, 28), (28, 32), (32, 36))
            with tc.tile_pool(name='cps', bufs=6, space='PSUM') as cps:
                for which in range(2):  # 0 = qk, 1 = v
                    w2 = w_qk2 if which == 0 else w_v2
                    w1p = w_qk1p if which == 0 else w_v1p
                    w1s = w_qk1s if which == 0 else w_v1s
                    for (ga, gb) in ((0, 4), (4, 8)):
                        psl = {g: cps.tile([128, 5, 96], F32, tag='c',
                                           name=f'c{which}_{g}')
                               for g in range(ga, gb)}
                        for tap in range(5):
                            for g in range(ga, gb):
                                r0, r1 = GRP[g]
                                sz = r1 - r0
                                if tap == 0:
                                    nc.tensor.matmul(
                                        psl[g][:, 0:sz, :], w2[:, 0, :],
                                        x2[:, r0:r1, 0:96],
                                        start=True, stop=False)
                                elif tap == 1:
                                    nc.tensor.matmul(
                                        psl[g][:, 0:sz, :], w2[:, 1, :],
                                        x2b[:, r0:r1, 0:96],
                                        start=False, stop=False)
                                elif tap == 2:
                                    nc.tensor.matmul(
                                        psl[g][:, 0:sz, :], w2[:, 2, :],
                                        x2[:, r0:r1, 2:98],
                                        start=False, stop=False)
                                elif tap == 3:
                                    nc.tensor.matmul(
                                        psl[g][:, 0:sz, :], w1p,
                                        xw[:, r0 + 2:r1 + 2, 0:96],
                                        start=False, stop=False)
                                else:
                                    nc.tensor.matmul(
                                        psl[g][:, 0:sz, :], w1s,
                                        x2[0:64, r0 + 2:r1 + 2, 2:98],
                                        start=False, stop=True)
                        for g in range(ga, gb):
                            r0, r1 = GRP[g]
                            sz = r1 - r0
                            if which == 0:
                                # k rows: all; q rows: kv rows 6..29 only
                                nc.vector.tensor_scalar(
                                    out=k_ext[0:64, r0 * 96:r1 * 96],
                                    in0=psl[g][64:128, 0:sz, :],
                                    scalar1=qkbias[64:128],
                                    scalar2=None, op0=mybir.AluOpType.add)
                                qa = max(r0, QOFF) - r0
                                qb = min(r1, QOFF + HS) - r0
                                if qa < qb:
                                    nc.scalar.activation(
                                        out=q_ext[0:64, (r0 + qa - QOFF) * 96:
                                                  (r0 + qb - QOFF) * 96],
                                        in_=psl[g][0:64, qa:qb, :],
                                        func=mybir.ActivationFunctionType.Identity,
                                        bias=qkbias[0:64])
                            else:
                                if g % 2 == 0:
                                    nc.scalar.activation(
                                        out=vsb[:, r0 * 96:r1 * 96],
                                        in_=psl[g][:, 0:sz, :],
                                        func=mybir.ActivationFunctionType.Identity,
                                        bias=vbias[:])
                                else:
                                    nc.vector.tensor_scalar(
                                        out=vsb[:, r0 * 96:r1 * 96],
                                        in0=psl[g][:, 0:sz, :],
                                        scalar1=vbias[:],
                                        scalar2=None, op0=mybir.AluOpType.add)

            # ----------------------------------- V^T + attention ----------
            wb_ap = wbT[:, :]

            with tc.tile_pool(name='tps', bufs=2, space='PSUM') as tps, \
                 tc.tile_pool(name='lps', bufs=2, space='PSUM') as lps, \
                 tc.tile_pool(name='ops', bufs=2, space='PSUM') as ops, \
                 tc.tile_pool(name='pxp', bufs=RING) as pxp, \
                 tc.tile_pool(name='att', bufs=3) as att:
                pexp = {}

                def vtrans(grp):
                    pst = tps.tile([96, 4, 128], BF16, tag='t')
                    for rr in range(4):
                        r = grp * 4 + rr
                        nc.tensor.transpose(
                            pst[:, rr, :], vsb[:, r * 96:(r + 1) * 96], ident)
                    if grp % 2 == 0:
                        nc.scalar.copy(vt[:, grp * 4:grp * 4 + 4, 0:128], pst[:])
                    else:
                        nc.vector.tensor_copy(
                            out=vt[:, grp * 4:grp * 4 + 4, 0:128], in_=pst[:])

                def logits(r):
                    j0, n = _users(r)
                    # rows padded to 128 f32 so 4 rows = one PSUM bank
                    # (matmul output cannot cross a bank boundary)
                    psL = lps.tile([96, 7, 128], F32, tag='L')
                    ke = k_ext[:, r * 96:(r + 1) * 96]
                    for (u0, u1) in ((0, min(n, 4)), (4, n)):
                        if u0 >= u1:
                            continue
                        qa = q_ext[:, (j0 + 2 * u0) * 96:(j0 + 2 * u0 + 1) * 96]
                        rhs = bass.AP(tensor=qa.tensor, offset=qa.offset,
                                      ap=[qa.ap[0], [2 * 96, u1 - u0], qa.ap[1]])
                        nc.tensor.matmul(psL[:, u0:u1, 0:96], ke, rhs,
                                         start=True, stop=True)
                    lm = att.tile([96, 7, 96], F32, tag='lm')
                    wb_b = bass.AP(tensor=wb_ap.tensor, offset=wb_ap.offset,
                                   ap=[wb_ap.ap[0], [0, n], wb_ap.ap[1]])
                    nc.vector.tensor_tensor(out=lm[:, 0:n, :],
                                            in0=psL[:, 0:n, 0:96],
                                            in1=wb_b, op=mybir.AluOpType.add)
                    px = pxp.tile([96, 7, 96], BF16, tag='px')
                    nc.scalar.activation(out=px[:, 0:n, :], in_=lm[:, 0:n, :],
                                         func=mybir.ActivationFunctionType.Exp)
                    pexp[r] = px

                def av(j):
                    psO = ops.tile([96, AVW], F32, tag='O')
                    for i in range(KS):
                        r = j + 2 * i
                        j0, _ = _users(r)
                        slot = (j - j0) // 2
                        nc.tensor.matmul(psO[:],
                                         pexp[r][:, slot, :],
                                         vt[:, r, 0:AVW],
                                         start=(i == 0), stop=(i == KS - 1))
                    rden = att.tile([96, 1], F32, tag='rden')
                    nc.vector.reciprocal(out=rden[:], in_=psO[:, 128:129])
                    oh = att.tile([96, 128], F32, tag='oh')
                    nc.vector.tensor_scalar_mul(oh[:], psO[:, 0:128], rden[:])
                    (nc.sync if j % 2 == 0 else nc.gpsimd).dma_start(
                        out=o[j], in_=oh[:])

                # interleave V^T transposes with the logits prologue so the
                # bias/exp pipeline is primed when the AV loop starts
                for grp in range(9):
                    vtrans(grp)
                    for r in ((2 * grp, 2 * grp + 1) if grp < 4 else
                              (grp + 4,)):
                        logits(r)
                for j in range(HS):
                    av(j)
                    if j + 13 < KV:
                        logits(j + 13)

    _split_excess_waits(nc)
    _CACHE['nc'] = nc
    return nc


# ---------------------------------------------------------------- kernel ---
def _make_in_maps(x, wq, bq, wk, bk, wv, bv):
    QD, KD = _hdist_channels()
    wbias = _wbias()
    wq_s = wq * SCALE
    w2 = np.zeros((3, 128, 128), dtype=np.float32)
    w1 = np.zeros((3, 64, 128), dtype=np.float32)
    v2 = np.zeros((3, 128, 128), dtype=np.float32)
    v1 = np.zeros((3, 64, 128), dtype=np.float32)
    for kx in range(3):
        w2[kx, 0:64, 0:64] = wq_s[:, :, 0, kx].T
        w2[kx, 0:64, 64:128] = wk[:, :, 0, kx].T
        w2[kx, 64:128, 0:64] = wq_s[:, :, 1, kx].T
        w2[kx, 64:128, 64:128] = wk[:, :, 1, kx].T
        w1[kx, :, 0:64] = wq_s[:, :, 2, kx].T
        w1[kx, :, 64:128] = wk[:, :, 2, kx].T
        v2[kx, 0:64, :] = wv[:, :, 0, kx].T
        v2[kx, 64:128, :] = wv[:, :, 1, kx].T
        v1[kx, :, :] = wv[:, :, 2, kx].T
    qkbias = np.concatenate([bq * SCALE, bk]).reshape(128, 1).astype(np.float32)
    vbias = bv.reshape(128, 1).astype(np.float32)

    in_maps = []
    for core in range(NCORES):
        b, slab = core // NH, core % NH
        h0 = slab * HS
        xsl = np.zeros((64, XR, 98), dtype=np.float32)
        r_lo, r_hi = h0 - 7, h0 - 7 + XR  # image rows of x slab
        src_lo, src_hi = max(0, r_lo), min(H, r_hi)
        xsl[:, src_lo - r_lo: src_hi - r_lo, 1:97] = x[b, :, src_lo:src_hi, :]
        qdf = np.repeat(QD[:, h0:h0 + HS, None], 96, axis=2).reshape(NDIST, -1)
        kdf = np.zeros((NDIST, KV, 96), dtype=np.float32)
        for r in range(KV):
            img = h0 - QOFF + r
            if 0 <= img < H:
                kdf[:, r, :] = KD[:, img, None]
        in_maps.append({
            'xs': xsl.astype(NPBF16),
            'wqk2': np.ascontiguousarray(w2.transpose(1, 0, 2)).astype(NPBF16),
            'wqk1': np.ascontiguousarray(w1.transpose(1, 0, 2)).astype(NPBF16),
            'wv2': np.ascontiguousarray(v2.transpose(1, 0, 2)).astype(NPBF16),
            'wv1': np.ascontiguousarray(v1.transpose(1, 0, 2)).astype(NPBF16),
            'qkb': qkbias, 'vb': vbias,
            'qd': np.ascontiguousarray(qdf).astype(NPBF16),
            'kd': np.ascontiguousarray(kdf.reshape(NDIST, -1)).astype(NPBF16),
            'wbt': np.ascontiguousarray(wbias.T),
        })
    return in_maps


def kernel(x, wq, bq, wk, bk, wv, bv):
    x = np.asarray(x, dtype=np.float32)
    wq = np.asarray(wq, dtype=np.float32)
    wk = np.asarray(wk, dtype=np.float32)
    wv = np.asarray(wv, dtype=np.float32)
    bq = np.asarray(bq, dtype=np.float32)
    bk = np.asarray(bk, dtype=np.float32)
    bv = np.asarray(bv, dtype=np.float32)

    nc = _build_program()
    in_maps = _make_in_maps(x=x, wq=wq, bq=bq, wk=wk, bk=bk, wv=wv, bv=bv)

    from concourse.bass_utils import run_bass_kernel_spmd
    res = run_bass_kernel_spmd(nc, in_maps, core_ids=list(range(NCORES)))
    global LAST_RESULT
    LAST_RESULT = res

    out = np.zeros((B, H, W, CO), dtype=np.float32)
    for core in range(NCORES):
        b, slab = core // NH, core % NH
        out[b, slab * HS:(slab + 1) * HS] = res.results[core]['o']

    border = _host_border(x, wq, bq, wk, bk, wv, bv)
    for h, val in border.items():
        out[:, h] = val.astype(np.float32)
    return out
